# revision 8
# baseline (speedup 1.0000x reference)
"""DetailAggregateLoss Trainium2 kernel.

Math (matches reference):
  g = gtmasks (0/1).  lap = 9*g - box3x3(g)  (3x3 laplacian via box sum).
  b = [lap >= 1] = g * [box3x3(g) <= 8]                     (full res)
  conv_s(g)[i,j] == conv_1(g)[s*i, s*j]  => bt_s = nearest-up of subsampled b
  fused = w0*b + w1*b@2-anchors + w2*b@4-anchors ; target = [fused > 0.1]
  bce  = mean(softplus(x) - x*target)          (softplus(x) = -ln(sigmoid(-x)))
  dice = mean_n(1 - (2*sum(p*t)+1)/(sum(p)+sum(t)+1)),  p = sigmoid(x) = 1 - s

Per-core (2 images), per 120-row tile:
  - g loaded as bf16 via strided DMA of the high half of each f32 (exact for 0/1),
    center rows at partitions 0..120, top-halo row at partition 121.
  - PE: psum_box = sum of 3 column-shifted tridiagonal matmuls (box sum);
        psum_f = w0*I@b + w1*R2@b_coldup2 + w2*R4@b_coldup4 (upsampled fuse sum)
  - DVE: b = (psum_box < 8.9) * g ;  (psum_f > mid)*s and (psum_f > mid)*x with
    f32 row-sum accumulators (the compare IS the target; never materialized)
  - ACT: s = sigmoid(-x) (accum -> sum s), sat-sigmoid of psum_f (accum -> sum t);
    phase B: ln(s) (accum -> -sum softplus)
Final scalar reduction on host in f64.
"""
import numpy as np
import ml_dtypes

import concourse.bacc as bacc
import concourse.tile as tile
import concourse.mybir as mybir
from concourse import bass_utils

F32 = mybir.dt.float32
BF16 = mybir.dt.bfloat16

B, H, W = 16, 1024, 1024
N_CORES = 8
IMGS = B // N_CORES          # images per core
TILE_R = 120                 # output rows per tile (multiple of 4)
ROW_TILES = [(t * TILE_R, min(TILE_R, H - t * TILE_R))
             for t in range((H + TILE_R - 1) // TILE_R)]  # 8x120 + 1x64
NT = len(ROW_TILES)
NSTAT = 5                    # per-tile stat columns: ssum, tsum, stsum, xtsum, lnsum
STAT_W = NT * NSTAT


def _fuse_threshold(fuse_kernel):
    """Pick the sat-sigmoid/is_gt threshold separating the 8 achievable
    hw fused values according to the reference f32 decision fused > 0.1."""
    w = np.asarray(fuse_kernel, dtype=np.float32).reshape(3)
    wb = w.astype(ml_dtypes.bfloat16).astype(np.float32)  # weights as PE sees them
    lo, hi = [], []
    for m in range(8):
        bits = [(m >> k) & 1 for k in range(3)]
        v_hw = np.float32(np.float32(wb[0] * bits[0] + wb[1] * bits[1])
                          + wb[2] * bits[2])
        v_ref = np.float32(np.float32(w[0] * bits[0] + w[1] * bits[1])
                           + w[2] * bits[2])
        (hi if v_ref > np.float32(0.1) else lo).append(v_hw)
    gap_lo, gap_hi = max(lo), min(hi)
    assert gap_hi > gap_lo + 1e-6, (gap_lo, gap_hi)
    mid = float((gap_lo + gap_hi) / 2.0)
    half = float((gap_hi - gap_lo) / 2.0)
    kk = min(250.0 / half, 1.0e6)
    return mid, kk, wb


def _const_matrices(wb):
    """lhsT constants (bf16). box: [122,120] tridiag with top halo at part 121.
    fuse: [120,120] w0*I, w1*R2(row anchors 2*(r//2)), w2*R4(4*(r//4))."""
    t3 = np.zeros((122, TILE_R), dtype=np.float32)
    for m in range(TILE_R):
        for k in (m - 1, m, m + 1):
            if k < 0:
                t3[121, m] = 1.0       # top halo row lives at partition 121
            else:
                t3[k, m] = 1.0
    w0i = np.zeros((TILE_R, TILE_R), dtype=np.float32)
    r2 = np.zeros((TILE_R, TILE_R), dtype=np.float32)
    r4 = np.zeros((TILE_R, TILE_R), dtype=np.float32)
    for r in range(TILE_R):
        w0i[r, r] = wb[0]
        r2[2 * (r // 2), r] = wb[1]
        r4[4 * (r // 4), r] = wb[2]
    bf = ml_dtypes.bfloat16
    return t3.astype(bf), w0i.astype(bf), r2.astype(bf), r4.astype(bf)


def _build(mid, kk):
    nc = bacc.Bacc("TRN2", target_bir_lowering=False, debug=False,
                   num_devices=N_CORES)
    x_in = nc.dram_tensor("x_in", (IMGS, H, W), F32, kind="ExternalInput")
    g_in = nc.dram_tensor("g_in", (IMGS, H, W), F32, kind="ExternalInput")
    t3_in = nc.dram_tensor("t3_in", (122, TILE_R), BF16, kind="ExternalInput")
    w0i_in = nc.dram_tensor("w0i_in", (TILE_R, TILE_R), BF16, kind="ExternalInput")
    r2_in = nc.dram_tensor("r2_in", (TILE_R, TILE_R), BF16, kind="ExternalInput")
    r4_in = nc.dram_tensor("r4_in", (TILE_R, TILE_R), BF16, kind="ExternalInput")
    zrow_in = nc.dram_tensor("zrow_in", (1, W + 2), BF16, kind="ExternalInput")
    stats_out = nc.dram_tensor("stats", (IMGS, TILE_R, STAT_W), F32,
                               kind="ExternalOutput")

    g16 = g_in[:].bitcast(mybir.dt.uint16)  # (IMGS, H, 2*W)

    with tile.TileContext(nc) as tc:
        with (
            tc.tile_pool(name="consts", bufs=1) as cpool,
            tc.tile_pool(name="g", bufs=3) as gpool,
            tc.tile_pool(name="x", bufs=3) as xpool,
            tc.tile_pool(name="b", bufs=2) as bpool,
            tc.tile_pool(name="s", bufs=IMGS * NT) as spool,
            tc.tile_pool(name="scr", bufs=2) as scrpool,
            tc.tile_pool(name="lnscr", bufs=2) as lnpool,
            tc.tile_pool(name="stats", bufs=IMGS) as statpool,
            tc.tile_pool(name="pbox", bufs=2, space="PSUM") as pbox_pool,
            tc.tile_pool(name="pfuse", bufs=2, space="PSUM") as pf_pool,
        ):
            t3 = cpool.tile([122, TILE_R], BF16)
            nc.sync.dma_start(t3[:], t3_in[:])
            w0i = cpool.tile([TILE_R, TILE_R], BF16)
            nc.sync.dma_start(w0i[:], w0i_in[:])
            r2 = cpool.tile([TILE_R, TILE_R], BF16)
            nc.sync.dma_start(r2[:], r2_in[:])
            r4 = cpool.tile([TILE_R, TILE_R], BF16)
            nc.sync.dma_start(r4[:], r4_in[:])
            sat_bias = cpool.tile([128, 1], F32)
            nc.gpsimd.memset(sat_bias[:], float(-kk * mid))

            stat_tiles = []
            s_tiles = [[None] * NT for _ in range(IMGS)]

            # ---------------- phase A ----------------
            for j in range(IMGS):
                stats = statpool.tile([TILE_R, STAT_W], F32)
                nc.gpsimd.memset(stats[:], 0.0)
                stat_tiles.append(stats)
                for t, (r0, rows) in enumerate(ROW_TILES):
                    g_bf = gpool.tile([122, W + 2], BF16)
                    gu16 = g_bf[:].bitcast(mybir.dt.uint16)
                    # main block: image rows r0..r0+rows(+1 bottom halo)
                    main_rows = min(rows + 1, H - r0)   # 121 normally, 64 for t8
                    for h in range(2):
                        c0 = 512 * h
                        nc.sync.dma_start(
                            gu16[0:main_rows, 1 + c0:1 + c0 + 512],
                            g16[j, r0:r0 + main_rows, 2 * c0 + 1:2 * (c0 + 512):2])
                    # zero column pads (both border cols, all partitions)
                    nc.gpsimd.memset(g_bf[:, 0:W + 2:W + 1], 0.0)
                    if r0 == 0:
                        nc.sync.dma_start(g_bf[121:122, :], zrow_in[:])
                    else:
                        nc.sync.dma_start(gu16[121:122, 1:W + 1],
                                          g16[j, r0 - 1:r0, 1::2])
                    if main_rows < rows + 1:
                        # bottom image edge: zero missing halo + stale slack
                        nc.gpsimd.memset(g_bf[main_rows:121, :], 0.0)

                    x_t = xpool.tile([TILE_R, W], F32)
                    for h in range(2):
                        c0 = 512 * h
                        nc.sync.dma_start(x_t[0:rows, c0:c0 + 512],
                                          x_in[j, r0:r0 + rows, c0:c0 + 512])

                    # box sum: 3 column-shifted tridiagonal matmuls
                    pbox = pbox_pool.tile([TILE_R, W], F32)
                    for h in range(2):
                        cs = slice(512 * h, 512 * h + 512)
                        for si, sh in enumerate((0, 1, 2)):
                            nc.tensor.matmul(
                                pbox[0:rows, cs], t3[:, 0:rows],
                                g_bf[:, sh + 512 * h: sh + 512 * h + 512],
                                start=(si == 0), stop=(si == 2))

                    # b = (box < 8.9) * g
                    b_t = bpool.tile([TILE_R, W], BF16)
                    nc.vector.scalar_tensor_tensor(
                        b_t[0:rows, :], pbox[0:rows, :], 8.9,
                        g_bf[0:rows, 1:W + 1],
                        op0=mybir.AluOpType.is_lt, op1=mybir.AluOpType.mult)

                    # fused = w0*b + w1*up2(b) + w2*up4(b)
                    pf = pf_pool.tile([TILE_R, W], F32)
                    for h in range(2):
                        cs = slice(512 * h, 512 * h + 512)
                        nc.tensor.matmul(pf[0:rows, cs], w0i[0:rows, 0:rows],
                                         b_t[0:rows, cs],
                                         start=True, stop=False)
                        ev = b_t[0:rows, 512 * h:512 * h + 512:2]
                        nc.tensor.matmul(pf[0:rows, cs], r2[0:rows, 0:rows],
                                         ev.unsqueeze(-1).broadcast_to((rows, 256, 2)),
                                         start=False, stop=False)
                        qv = b_t[0:rows, 512 * h:512 * h + 512:4]
                        nc.tensor.matmul(pf[0:rows, cs], r4[0:rows, 0:rows],
                                         qv.unsqueeze(-1).broadcast_to((rows, 128, 4)),
                                         start=False, stop=True)

                    # s = sigmoid(-x), f32 (accum: sum s)
                    s_t = spool.tile([TILE_R, W], F32)
                    s_tiles[j][t] = s_t
                    nc.scalar.activation(
                        s_t[0:rows, :], x_t[0:rows, :],
                        mybir.ActivationFunctionType.Sigmoid, scale=-1.0,
                        accum_out=stats[0:rows, t * NSTAT + 0: t * NSTAT + 1])

                    # target row-sums via saturated sigmoid of fused
                    t_scr = scrpool.tile([TILE_R, W], BF16, tag="tscr")
                    nc.scalar.activation(
                        t_scr[0:rows, :], pf[0:rows, :],
                        mybir.ActivationFunctionType.Sigmoid,
                        scale=float(kk), bias=sat_bias[0:rows, :],
                        accum_out=stats[0:rows, t * NSTAT + 1: t * NSTAT + 2])

                    # sum s*t and sum x*t (the compare IS the target)
                    st_scr = scrpool.tile([TILE_R, W], BF16, tag="stscr")
                    nc.vector.scalar_tensor_tensor(
                        st_scr[0:rows, :], pf[0:rows, :], float(mid),
                        s_t[0:rows, :],
                        op0=mybir.AluOpType.is_gt, op1=mybir.AluOpType.mult,
                        accum_out=stats[0:rows, t * NSTAT + 2: t * NSTAT + 3])
                    xt_scr = scrpool.tile([TILE_R, W], BF16, tag="xtscr")
                    nc.vector.scalar_tensor_tensor(
                        xt_scr[0:rows, :], pf[0:rows, :], float(mid),
                        x_t[0:rows, :],
                        op0=mybir.AluOpType.is_gt, op1=mybir.AluOpType.mult,
                        accum_out=stats[0:rows, t * NSTAT + 3: t * NSTAT + 4])

            # ---------------- phase B: ln(s) ----------------
            for j in range(IMGS):
                for t, (r0, rows) in enumerate(ROW_TILES):
                    ln_scr = lnpool.tile([TILE_R, W], F32)
                    nc.scalar.activation(
                        ln_scr[0:rows, :], s_tiles[j][t][0:rows, :],
                        mybir.ActivationFunctionType.Ln,
                        accum_out=stat_tiles[j][0:rows,
                                               t * NSTAT + 4: t * NSTAT + 5])
                nc.sync.dma_start(stats_out[j], stat_tiles[j][:])

    nc.compile()
    return nc


_CACHE = {}


def _get_nc(mid, kk):
    key = (round(mid, 9), round(kk, 3))
    if key not in _CACHE:
        _CACHE[key] = _build(mid, kk)
    return _CACHE[key]


def kernel(boundary_logits, gtmasks, fuse_kernel):
    x = np.ascontiguousarray(np.asarray(boundary_logits, dtype=np.float32)
                             .reshape(B, H, W))
    g = np.ascontiguousarray(np.asarray(gtmasks, dtype=np.float32)
                             .reshape(B, H, W))
    mid, kk, wb = _fuse_threshold(fuse_kernel)
    t3, w0i, r2, r4 = _const_matrices(wb)
    nc = _get_nc(mid, kk)

    in_maps = []
    for c in range(N_CORES):
        sl = slice(c * IMGS, (c + 1) * IMGS)
        in_maps.append({
            "x_in": np.ascontiguousarray(x[sl]),
            "g_in": np.ascontiguousarray(g[sl]),
            "t3_in": t3, "w0i_in": w0i, "r2_in": r2, "r4_in": r4,
            "zrow_in": np.zeros((1, W + 2), dtype=ml_dtypes.bfloat16),
        })
    res = bass_utils.run_bass_kernel_spmd(nc, in_maps,
                                          core_ids=list(range(N_CORES)))

    n = float(H * W)
    bce_num = 0.0
    dice_sum = 0.0
    for c in range(N_CORES):
        stats = res.results[c]["stats"].astype(np.float64)  # [IMGS, 120, STAT_W]
        for j in range(IMGS):
            st = stats[j]
            ssum = st[:, 0::NSTAT].sum()
            tsum = st[:, 1::NSTAT].sum()
            stsum = st[:, 2::NSTAT].sum()
            xtsum = st[:, 3::NSTAT].sum()
            lnsum = st[:, 4::NSTAT].sum()
            psum = n - ssum
            ptsum = tsum - stsum
            softplus_sum = -lnsum
            bce_num += softplus_sum - xtsum
            dice_sum += 1.0 - (2.0 * ptsum + 1.0) / (psum + tsum + 1.0)
    bce = np.float32(bce_num / (B * n))
    dice = np.float32(dice_sum / B)
    return bce, dice


# revision 10
# speedup vs baseline: 1.5987x; 1.5987x over previous
"""DetailAggregateLoss Trainium2 kernel.

Math (matches reference):
  g = gtmasks (0/1).  lap = 9*g - box3x3(g)  (3x3 laplacian via box sum).
  b = [lap >= 1] = g * [box3x3(g) <= 8]                     (full res)
  conv_s(g)[i,j] == conv_1(g)[s*i, s*j]  => bt_s = nearest-up of subsampled b
  fused = w0*b + w1*b@2-anchors + w2*b@4-anchors ; target = [fused > 0.1]
  bce  = mean(softplus(x) - x*target)          (softplus(x) = -ln(sigmoid(-x)))
  dice = mean_n(1 - (2*sum(p*t)+1)/(sum(p)+sum(t)+1)),  p = sigmoid(x) = 1 - s

Per-core (2 images), per 120-row tile:
  - g loaded as bf16 via strided DMA of the high half of each f32 (exact for 0/1),
    center rows at partitions 0..120, top-halo row at partition 121.
  - PE: psum_box = sum of 3 column-shifted tridiagonal matmuls (box sum);
        psum_f = w0*I@b + w1*R2@b_coldup2 + w2*R4@b_coldup4 (upsampled fuse sum)
  - DVE: b = (psum_box < 8.9) * g ;  (psum_f > mid)*s and (psum_f > mid)*x with
    f32 row-sum accumulators (the compare IS the target; never materialized)
  - ACT: s = sigmoid(-x) (accum -> sum s), sat-sigmoid of psum_f (accum -> sum t);
    phase B: ln(s) (accum -> -sum softplus)
Final scalar reduction on host in f64.
"""
import numpy as np
import ml_dtypes
import jax

import concourse.bacc as bacc
import concourse.tile as tile
import concourse.mybir as mybir
from concourse import bass2jax

F32 = mybir.dt.float32
BF16 = mybir.dt.bfloat16

B, H, W = 16, 1024, 1024
N_CORES = 8
IMGS = B // N_CORES          # images per core
TILE_R = 120                 # output rows per tile (multiple of 4)
ROW_TILES = [(t * TILE_R, min(TILE_R, H - t * TILE_R))
             for t in range((H + TILE_R - 1) // TILE_R)]  # 8x120 + 1x64
NT = len(ROW_TILES)
NSTAT = 5                    # per-tile stat columns: ssum, tsum, stsum, xtsum, lnsum
STAT_W = NT * NSTAT


def _fuse_threshold(fuse_kernel):
    """Pick the sat-sigmoid/is_gt threshold separating the 8 achievable
    hw fused values according to the reference f32 decision fused > 0.1."""
    w = np.asarray(fuse_kernel, dtype=np.float32).reshape(3)
    wb = w.astype(ml_dtypes.bfloat16).astype(np.float32)  # weights as PE sees them
    lo, hi = [], []
    for m in range(8):
        bits = [(m >> k) & 1 for k in range(3)]
        v_hw = np.float32(np.float32(wb[0] * bits[0] + wb[1] * bits[1])
                          + wb[2] * bits[2])
        v_ref = np.float32(np.float32(w[0] * bits[0] + w[1] * bits[1])
                           + w[2] * bits[2])
        (hi if v_ref > np.float32(0.1) else lo).append(v_hw)
    gap_lo, gap_hi = max(lo), min(hi)
    assert gap_hi > gap_lo + 1e-6, (gap_lo, gap_hi)
    mid = float((gap_lo + gap_hi) / 2.0)
    half = float((gap_hi - gap_lo) / 2.0)
    kk = min(250.0 / half, 1.0e6)
    return mid, kk, wb


def _const_matrices(wb):
    """lhsT constants (bf16). box: [122,120] tridiag with top halo at part 121.
    fuse: [120,120] w0*I, w1*R2(row anchors 2*(r//2)), w2*R4(4*(r//4))."""
    t3 = np.zeros((122, TILE_R), dtype=np.float32)
    for m in range(TILE_R):
        for k in (m - 1, m, m + 1):
            if k < 0:
                t3[121, m] = 1.0       # top halo row lives at partition 121
            else:
                t3[k, m] = 1.0
    w0i = np.zeros((TILE_R, TILE_R), dtype=np.float32)
    r2 = np.zeros((TILE_R, TILE_R), dtype=np.float32)
    r4 = np.zeros((TILE_R, TILE_R), dtype=np.float32)
    for r in range(TILE_R):
        w0i[r, r] = wb[0]
        r2[2 * (r // 2), r] = wb[1]
        r4[4 * (r // 4), r] = wb[2]
    bf = ml_dtypes.bfloat16
    return t3.astype(bf), w0i.astype(bf), r2.astype(bf), r4.astype(bf)


def _build(mid, kk):
    nc = bacc.Bacc("TRN2", target_bir_lowering=False, debug=False,
                   num_devices=N_CORES)
    x_in = nc.dram_tensor("x_in", (IMGS, H, W), F32, kind="ExternalInput")
    g_in = nc.dram_tensor("g_in", (IMGS, H, W), F32, kind="ExternalInput")
    t3_in = nc.dram_tensor("t3_in", (122, TILE_R), BF16, kind="ExternalInput")
    w0i_in = nc.dram_tensor("w0i_in", (TILE_R, TILE_R), BF16, kind="ExternalInput")
    r2_in = nc.dram_tensor("r2_in", (TILE_R, TILE_R), BF16, kind="ExternalInput")
    r4_in = nc.dram_tensor("r4_in", (TILE_R, TILE_R), BF16, kind="ExternalInput")
    zrow_in = nc.dram_tensor("zrow_in", (1, W + 2), BF16, kind="ExternalInput")
    stats_out = nc.dram_tensor("stats", (IMGS, TILE_R, STAT_W), F32,
                               kind="ExternalOutput")

    g16 = g_in[:].bitcast(mybir.dt.uint16)  # (IMGS, H, 2*W)

    with tile.TileContext(nc) as tc:
        with (
            tc.tile_pool(name="consts", bufs=1) as cpool,
            tc.tile_pool(name="g", bufs=3) as gpool,
            tc.tile_pool(name="x", bufs=3) as xpool,
            tc.tile_pool(name="b", bufs=2) as bpool,
            tc.tile_pool(name="s", bufs=IMGS * NT) as spool,
            tc.tile_pool(name="scr", bufs=2) as scrpool,
            tc.tile_pool(name="lnscr", bufs=2) as lnpool,
            tc.tile_pool(name="stats", bufs=IMGS) as statpool,
            tc.tile_pool(name="pbox", bufs=2, space="PSUM") as pbox_pool,
            tc.tile_pool(name="pfuse", bufs=2, space="PSUM") as pf_pool,
        ):
            t3 = cpool.tile([122, TILE_R], BF16)
            nc.sync.dma_start(t3[:], t3_in[:])
            w0i = cpool.tile([TILE_R, TILE_R], BF16)
            nc.sync.dma_start(w0i[:], w0i_in[:])
            r2 = cpool.tile([TILE_R, TILE_R], BF16)
            nc.sync.dma_start(r2[:], r2_in[:])
            r4 = cpool.tile([TILE_R, TILE_R], BF16)
            nc.sync.dma_start(r4[:], r4_in[:])
            sat_bias = cpool.tile([128, 1], F32)
            nc.gpsimd.memset(sat_bias[:], float(-kk * mid))

            stat_tiles = []
            s_tiles = [[None] * NT for _ in range(IMGS)]

            # ---------------- phase A ----------------
            for j in range(IMGS):
                stats = statpool.tile([TILE_R, STAT_W], F32)
                nc.gpsimd.memset(stats[:], 0.0)
                stat_tiles.append(stats)
                for t, (r0, rows) in enumerate(ROW_TILES):
                    g_bf = gpool.tile([122, W + 2], BF16)
                    gu16 = g_bf[:].bitcast(mybir.dt.uint16)
                    # main block: image rows r0..r0+rows(+1 bottom halo)
                    main_rows = min(rows + 1, H - r0)   # 121 normally, 64 for t8
                    for h in range(2):
                        c0 = 512 * h
                        nc.sync.dma_start(
                            gu16[0:main_rows, 1 + c0:1 + c0 + 512],
                            g16[j, r0:r0 + main_rows, 2 * c0 + 1:2 * (c0 + 512):2])
                    # zero column pads (both border cols, all partitions)
                    nc.gpsimd.memset(g_bf[:, 0:W + 2:W + 1], 0.0)
                    if r0 == 0:
                        nc.sync.dma_start(g_bf[121:122, :], zrow_in[:])
                    else:
                        nc.sync.dma_start(gu16[121:122, 1:W + 1],
                                          g16[j, r0 - 1:r0, 1::2])
                    if main_rows < rows + 1:
                        # bottom image edge: zero missing halo + stale slack
                        nc.gpsimd.memset(g_bf[main_rows:121, :], 0.0)

                    x_t = xpool.tile([TILE_R, W], F32)
                    for h in range(2):
                        c0 = 512 * h
                        nc.sync.dma_start(x_t[0:rows, c0:c0 + 512],
                                          x_in[j, r0:r0 + rows, c0:c0 + 512])

                    # box sum: 3 column-shifted tridiagonal matmuls
                    pbox = pbox_pool.tile([TILE_R, W], F32)
                    for h in range(2):
                        cs = slice(512 * h, 512 * h + 512)
                        for si, sh in enumerate((0, 1, 2)):
                            nc.tensor.matmul(
                                pbox[0:rows, cs], t3[:, 0:rows],
                                g_bf[:, sh + 512 * h: sh + 512 * h + 512],
                                start=(si == 0), stop=(si == 2))

                    # b = (box < 8.9) * g
                    b_t = bpool.tile([TILE_R, W], BF16)
                    nc.vector.scalar_tensor_tensor(
                        b_t[0:rows, :], pbox[0:rows, :], 8.9,
                        g_bf[0:rows, 1:W + 1],
                        op0=mybir.AluOpType.is_lt, op1=mybir.AluOpType.mult)

                    # fused = w0*b + w1*up2(b) + w2*up4(b)
                    pf = pf_pool.tile([TILE_R, W], F32)
                    for h in range(2):
                        cs = slice(512 * h, 512 * h + 512)
                        nc.tensor.matmul(pf[0:rows, cs], w0i[0:rows, 0:rows],
                                         b_t[0:rows, cs],
                                         start=True, stop=False)
                        ev = b_t[0:rows, 512 * h:512 * h + 512:2]
                        nc.tensor.matmul(pf[0:rows, cs], r2[0:rows, 0:rows],
                                         ev.unsqueeze(-1).broadcast_to((rows, 256, 2)),
                                         start=False, stop=False)
                        qv = b_t[0:rows, 512 * h:512 * h + 512:4]
                        nc.tensor.matmul(pf[0:rows, cs], r4[0:rows, 0:rows],
                                         qv.unsqueeze(-1).broadcast_to((rows, 128, 4)),
                                         start=False, stop=True)

                    # s = sigmoid(-x), f32 (accum: sum s)
                    s_t = spool.tile([TILE_R, W], F32)
                    s_tiles[j][t] = s_t
                    nc.scalar.activation(
                        s_t[0:rows, :], x_t[0:rows, :],
                        mybir.ActivationFunctionType.Sigmoid, scale=-1.0,
                        accum_out=stats[0:rows, t * NSTAT + 0: t * NSTAT + 1])

                    # target row-sums via saturated sigmoid of fused
                    t_scr = scrpool.tile([TILE_R, W], BF16, tag="tscr")
                    nc.scalar.activation(
                        t_scr[0:rows, :], pf[0:rows, :],
                        mybir.ActivationFunctionType.Sigmoid,
                        scale=float(kk), bias=sat_bias[0:rows, :],
                        accum_out=stats[0:rows, t * NSTAT + 1: t * NSTAT + 2])

                    # sum s*t and sum x*t (the compare IS the target)
                    st_scr = scrpool.tile([TILE_R, W], BF16, tag="stscr")
                    nc.vector.scalar_tensor_tensor(
                        st_scr[0:rows, :], pf[0:rows, :], float(mid),
                        s_t[0:rows, :],
                        op0=mybir.AluOpType.is_gt, op1=mybir.AluOpType.mult,
                        accum_out=stats[0:rows, t * NSTAT + 2: t * NSTAT + 3])
                    xt_scr = scrpool.tile([TILE_R, W], BF16, tag="xtscr")
                    nc.vector.scalar_tensor_tensor(
                        xt_scr[0:rows, :], pf[0:rows, :], float(mid),
                        x_t[0:rows, :],
                        op0=mybir.AluOpType.is_gt, op1=mybir.AluOpType.mult,
                        accum_out=stats[0:rows, t * NSTAT + 3: t * NSTAT + 4])

            # ---------------- phase B: ln(s) ----------------
            for j in range(IMGS):
                for t, (r0, rows) in enumerate(ROW_TILES):
                    ln_scr = lnpool.tile([TILE_R, W], F32)
                    nc.scalar.activation(
                        ln_scr[0:rows, :], s_tiles[j][t][0:rows, :],
                        mybir.ActivationFunctionType.Ln,
                        accum_out=stat_tiles[j][0:rows,
                                               t * NSTAT + 4: t * NSTAT + 5])
                nc.sync.dma_start(stats_out[j], stat_tiles[j][:])

    nc.compile()
    return nc


def _make_runner(nc):
    """Cached 8-core shard_map runner (mirrors bass2jax.run_bass_via_pjrt but
    traces/compiles the jit wrapper once)."""
    bass2jax.install_neuronx_cc_hook()
    partition_name = (nc.partition_id_tensor.name
                      if nc.partition_id_tensor else None)
    in_names, out_names, out_avals = [], [], []
    for alloc in nc.m.functions[0].allocations:
        if not isinstance(alloc, mybir.MemoryLocationSet):
            continue
        name = alloc.memorylocations[0].name
        if alloc.kind == "ExternalInput":
            if name != partition_name:
                in_names.append(name)
        elif alloc.kind == "ExternalOutput":
            out_names.append(name)
            out_avals.append(jax.core.ShapedArray(
                tuple(alloc.tensor_shape), mybir.dt.np(alloc.dtype)))
    n_params = len(in_names)
    all_names = in_names + out_names
    if partition_name is not None:
        all_names.append(partition_name)
    donate = tuple(range(n_params, n_params + len(out_names)))

    def _body(*args):
        operands = list(args)
        if partition_name is not None:
            operands.append(bass2jax.partition_id_tensor())
        return tuple(bass2jax._bass_exec_p.bind(
            *operands,
            out_avals=tuple(out_avals),
            in_names=tuple(all_names),
            out_names=tuple(out_names),
            lowering_input_output_aliases=(),
            sim_require_finite=True,
            sim_require_nnan=True,
            nc=nc,
        ))

    devices = jax.devices()[:N_CORES]
    mesh = bass2jax.Mesh(np.asarray(devices), ("core",))
    in_specs = (bass2jax.PartitionSpec("core"),) * (n_params + len(out_names))
    out_specs = (bass2jax.PartitionSpec("core"),) * len(out_names)
    sharded = jax.jit(
        bass2jax.shard_map(_body, mesh=mesh, in_specs=in_specs,
                           out_specs=out_specs, check_rep=False),
        donate_argnums=donate, keep_unused=True)
    return sharded, in_names, out_names, out_avals


_CACHE = {}


def _get_runner(mid, kk):
    key = (round(mid, 9), round(kk, 3))
    if key not in _CACHE:
        nc = _build(mid, kk)
        _CACHE[key] = _make_runner(nc)
    return _CACHE[key]


def _run_device(x, g, mid, kk, wb):
    """x, g: (B, H, W) f32 host arrays. Returns stats (N_CORES, IMGS, TILE_R, STAT_W)."""
    sharded, in_names, out_names, out_avals = _get_runner(mid, kk)
    t3, w0i, r2, r4 = _const_matrices(wb)
    glob = {
        "x_in": x, "g_in": g,
        "t3_in": np.tile(t3, (N_CORES, 1)),
        "w0i_in": np.tile(w0i, (N_CORES, 1)),
        "r2_in": np.tile(r2, (N_CORES, 1)),
        "r4_in": np.tile(r4, (N_CORES, 1)),
        "zrow_in": np.zeros((N_CORES, W + 2), dtype=ml_dtypes.bfloat16),
    }
    args = [glob[name] for name in in_names]
    args += [np.zeros((N_CORES * a.shape[0], *a.shape[1:]), a.dtype)
             for a in out_avals]
    outs = sharded(*args)
    i = out_names.index("stats")
    return (np.asarray(outs[i])
            .reshape(N_CORES, IMGS, TILE_R, STAT_W).astype(np.float64))


def kernel(boundary_logits, gtmasks, fuse_kernel):
    x = np.asarray(boundary_logits, dtype=np.float32).reshape(B, H, W)
    g = np.asarray(gtmasks, dtype=np.float32).reshape(B, H, W)
    mid, kk, wb = _fuse_threshold(fuse_kernel)
    stats = _run_device(x, g, mid, kk, wb)

    n = float(H * W)
    bce_num = 0.0
    dice_sum = 0.0
    for c in range(N_CORES):
        for j in range(IMGS):
            st = stats[c, j]
            ssum = st[:, 0::NSTAT].sum()
            tsum = st[:, 1::NSTAT].sum()
            stsum = st[:, 2::NSTAT].sum()
            xtsum = st[:, 3::NSTAT].sum()
            lnsum = st[:, 4::NSTAT].sum()
            psum = n - ssum
            ptsum = tsum - stsum
            bce_num += -lnsum - xtsum
            dice_sum += 1.0 - (2.0 * ptsum + 1.0) / (psum + tsum + 1.0)
    bce = np.float32(bce_num / (B * n))
    dice = np.float32(dice_sum / B)
    return bce, dice
